# revision 8
# baseline (speedup 1.0000x reference)
"""DVGCL (GNN message passing + contrastive losses) on 8 Trainium2 cores.

v2. Sharding: node dim N split 8 ways by destination; each shard degree-sorted
and laid out cyclically (pos j -> partition j%128, col j//128); the permutation
is folded into every index array on the host. Propagation gathers the
pre-scaled bf16 table y = d_inv * cur via per-(tile,slot) indirect DMAs with
CCE-add accumulation (one 128-row gather per op is the HW limit; ~1.2us/op of
Q7 descriptor-gen is the floor). Sentinel slots point at a dedicated zero row
of the table, so no memsets or bounds checks are needed.

Layer pipelining: each layer's y table is AllGathered in NPIECE column-range
pieces; gathers are issued piece-major so piece p's scale+write+collective
overlaps piece p+1's gathers, leaving only the last piece's collective on the
critical path at each layer boundary.

The final 38.6MB all_emb AllGather is replaced by a 2-phase batch path: each
core indirect-gathers the ~B*3/8 batch rows it owns from its local shard into
a compact buffer, one small AllGather shares those, and 68 assembly gathers
rebuild the full-batch tables per core. Intent pipes are data-parallel (each
core computes its 1/8 of the batch; normalized transposed slices are
AllGathered for the InfoNCE negatives). eps rows are pre-gathered on the host;
the emb/int regularizer losses are pure functions of the inputs and are
computed on the host in combine().

Walrus codegen accepts at most ONE sync wait per instruction, so
split_multi_waits hoists extras onto same-engine NoOps after Tile scheduling.
"""
import math
import numpy as np

import concourse.bass as bass
import concourse.mybir as mybir
import concourse.tile as tile
from concourse.bass_utils import run_bass_kernel_spmd
from concourse.masks import make_identity

F32 = mybir.dt.float32
BF16 = mybir.dt.bfloat16
I32 = mybir.dt.int32
AX = mybir.AxisListType
ALU = mybir.AluOpType
ACTF = mybir.ActivationFunctionType


def default_cfg():
    return dict(
        N_USERS=50000, N_ITEMS=100000, D=64, N_LAYERS=3, N_INTENTS=128,
        T_SIZE=32, TEMP=0.2, KL_REG=0.01, EMB_REG=1e-5, INT_REG=1e-5,
        SSL_REG=0.1, B=4096, NC=8, NPIECE=8,
    )


def derive(cfg):
    c = dict(cfg)
    c["N"] = c["N_USERS"] + c["N_ITEMS"]
    assert c["N"] % c["NC"] == 0
    c["SHARD"] = c["N"] // c["NC"]
    c["PC"] = math.ceil(c["SHARD"] / 128)
    c["SPAD"] = 128 * c["PC"]
    c["TROWS"] = c["NC"] * c["SPAD"]
    assert c["B"] % 128 == 0 and (c["B"] // c["NC"]) % 128 == 0
    c["BCOLS"] = c["B"] // 128
    c["BSH"] = c["B"] // c["NC"]
    c["BSHC"] = c["BSH"] // 128
    # piece column ranges (NPIECE ranges over PC columns)
    w = math.ceil(c["PC"] / c["NPIECE"])
    edges = [min(p * w, c["PC"]) for p in range(c["NPIECE"] + 1)]
    c["PIECES"] = [(edges[p], edges[p + 1]) for p in range(c["NPIECE"])
                   if edges[p + 1] > edges[p]]
    return c


# --------------------------------------------------------------------------
# wait splitting post-pass (walrus: max 1 sync wait per instruction)
# --------------------------------------------------------------------------

def split_multi_waits(nc, max_waits=1):
    n = 0
    for f in nc.m.functions:
        for b in f.blocks:
            insts = b.instructions
            items = list(insts)
            out = []
            for i in items:
                si = i.sync_info
                w = list(si.on_wait) if si and si.on_wait else []
                if len(w) > max_waits:
                    for x in w[:-max_waits]:
                        n += 1
                        out.append(mybir.InstNoOp(
                            name=f"waitsplit-{n}",
                            sync_info=mybir.SyncInfo(on_wait=[x], on_update=[]),
                            engine=i.engine, bass_nofuse=True))
                    si.on_wait = w[-max_waits:]
                out.append(i)
            insts.clear()
            insts.extend(out)
    return n


# --------------------------------------------------------------------------
# host prep
# --------------------------------------------------------------------------

def host_prep(inputs, c):
    N, NC, SHARD, SPAD, PC, D = (c["N"], c["NC"], c["SHARD"], c["SPAD"],
                                 c["PC"], c["D"])
    h = np.asarray(inputs["h_list"]).astype(np.int64)
    t = np.asarray(inputs["t_list"]).astype(np.int64)

    deg = np.bincount(h, minlength=N).astype(np.int64)
    with np.errstate(divide="ignore"):
        d_inv = (deg.astype(np.float64) ** -0.5).astype(np.float32)

    # perm position j of node n: shard k = n // SHARD, degree-sorted inside
    perm_pos = np.empty(N, dtype=np.int64)   # node -> (core, j)
    inv_order = []
    for k in range(NC):
        lo = k * SHARD
        order = np.argsort(deg[lo:lo + SHARD], kind="stable")
        perm_pos[lo + order] = k * SPAD + np.arange(SHARD)
        inv_order.append(lo + order)

    # piece-major AllGather table layout: piece p holds cols [c0, c1) of all
    # cores, core-major inside the piece. trow(k, j):
    pieces = c["PIECES"]
    piece_of_col = np.empty(PC, dtype=np.int64)
    piece_base = np.empty(PC, dtype=np.int64)   # table row base of col's piece
    col_in_piece = np.empty(PC, dtype=np.int64)
    base = 0
    for (c0, c1) in pieces:
        piece_of_col[c0:c1] = base
        for cc in range(c0, c1):
            piece_base[cc] = base
            col_in_piece[cc] = cc - c0
        base += NC * (c1 - c0) * 128
    TROWS_TBL = base  # == NC * SPAD

    def trow_of_pos(pos):
        """global perm position (k*SPAD + j) -> table row in piece-major"""
        k = pos // SPAD
        j = pos % SPAD
        p128 = j % 128
        cc = j // 128
        (w0,) = (piece_of_col[cc],)
        c0 = cc - col_in_piece[cc]
        # width of the piece this col is in
        w = None
        for (a, b) in pieces:
            if a <= cc < b:
                w = b - a
                break
        return (piece_base[cc] + k * w * 128 + (cc - c0) * 128 + p128)

    # vectorized trow
    piece_w = np.empty(PC, dtype=np.int64)
    piece_c0 = np.empty(PC, dtype=np.int64)
    for (a, b) in pieces:
        piece_w[a:b] = b - a
        piece_c0[a:b] = a

    def trow_vec(pos):
        pos = np.asarray(pos, dtype=np.int64)
        k = pos // SPAD
        j = pos % SPAD
        p128 = j % 128
        cc = j // 128
        return (piece_base[cc] + k * piece_w[cc] * 128
                + (cc - piece_c0[cc]) * 128 + p128)

    ZROW = TROWS_TBL  # dedicated zero row

    dest_pos = perm_pos[h]
    eorder = np.argsort(dest_pos, kind="stable")
    dpos_s = dest_pos[eorder]
    src_rows = trow_vec(perm_pos[t[eorder]])

    ego = np.concatenate([
        np.asarray(inputs["user_emb"], dtype=np.float32),
        np.asarray(inputs["item_emb"], dtype=np.float32),
    ], axis=0)

    # per-core per-tile slot columns (values = table rows; pad = ZROW)
    core_cols = []       # list of dict[(tau, s)] -> int64[128]
    core_smax = []
    for k in range(NC):
        basek = k * SPAD
        lo_i = np.searchsorted(dpos_s, basek)
        hi_i = np.searchsorted(dpos_s, basek + SHARD)
        dj = dpos_s[lo_i:hi_i] - basek
        sj = src_rows[lo_i:hi_i]
        degl = np.zeros(SPAD, dtype=np.int64)
        np.add.at(degl, dj, 1)
        starts = np.zeros(SPAD + 1, dtype=np.int64)
        np.cumsum(degl, out=starts[1:])
        cols = {}
        smax = np.zeros(PC, dtype=np.int64)
        for tau in range(PC):
            jlo = tau * 128
            dtile = degl[jlo:jlo + 128]
            smax[tau] = int(dtile.max())
            for s in range(smax[tau]):
                col = np.full(128, ZROW, dtype=np.int64)
                sel = dtile > s
                col[sel] = sj[starts[jlo:jlo + 128][sel] + s]
                cols[(tau, s)] = col
        core_cols.append(cols)
        core_smax.append(smax)

    # SPMD union plan: per tile, slot count = max across cores (>=1 so the
    # first op's unconditional write covers pad lanes with zeros)
    smax_u = np.maximum(np.max(np.stack(core_smax), axis=0), 1)
    NI = int(smax_u.sum())
    c["SMAX_U"] = smax_u
    c["NI"] = NI

    # idx_spmm layout: piece-major, within piece slot-major round-robin over
    # that piece's tiles. colof[(tau, s)] -> column index in idx_spmm.
    colof = {}
    _ic = 0
    order_ops = []
    for (c0, c1) in c["PIECES"]:
        sm = int(smax_u[c0:c1].max())
        for s in range(sm):
            for tau in range(c0, c1):
                if s < smax_u[tau]:
                    colof[(tau, s)] = _ic
                    order_ops.append((tau, s))
                    _ic += 1
    assert _ic == NI
    c["COLOF"] = colof
    c["OP_ORDER"] = order_ops

    per_core = []
    for k in range(NC):
        idx = np.full((128, NI), ZROW, dtype=np.int32)
        for (tau, s), col in colof.items():
            v = core_cols[k].get((tau, s))
            if v is not None:
                idx[:, col] = v
        def cyc(vec):
            return vec.reshape(PC, 128).T.copy()
        dloc = np.zeros(SPAD, dtype=np.float32)
        dloc[:SHARD] = d_inv[inv_order[k]]
        mask = np.zeros(SPAD, dtype=np.float32)
        mask[:SHARD] = 1.0
        egp = np.zeros((SPAD, D), dtype=np.float32)
        egp[:SHARD] = ego[inv_order[k]]
        per_core.append(dict(
            idx_spmm=idx, dinv=cyc(dloc), dinv2=cyc(dloc * dloc),
            kmask=cyc(mask), ego_perm=egp))

    return per_core, perm_pos, trow_vec, ZROW


# --------------------------------------------------------------------------
# device program
# --------------------------------------------------------------------------

def build_bass(c):
    NC, D, PC, SPAD, TROWS = c["NC"], c["D"], c["PC"], c["SPAD"], c["TROWS"]
    BC, BSHC, NI, SK = c["BCOLS"], c["BSHC"], c["NI"], c["SK"]
    NINT, TS, NL = c["N_INTENTS"], c["T_SIZE"], c["N_LAYERS"]
    TEMP = c["TEMP"]
    smax_u = c["SMAX_U"]
    colof = c["COLOF"]
    pieces = c["PIECES"]
    # assembled batch tile columns: IU (BC) | IP (BC) | NS (BSHC)
    NB = 2 * BC + BSHC
    OFF_IU, OFF_IP, OFF_NS = 0, BC, 2 * BC

    nc = bass.Bass(num_devices=NC)

    ego_perm = nc.dram_tensor("ego_perm", [SPAD, D], F32, kind="ExternalInput")
    idx_spmm = nc.dram_tensor("idx_spmm", [128, NI], I32, kind="ExternalInput")
    dinv_in = nc.dram_tensor("dinv", [128, PC], F32, kind="ExternalInput")
    dinv2_in = nc.dram_tensor("dinv2", [128, PC], F32, kind="ExternalInput")
    kmask_in = nc.dram_tensor("kmask", [128, PC], F32, kind="ExternalInput")
    idx_own = nc.dram_tensor("idx_own", [128, SK], I32, kind="ExternalInput")
    idx_asm = nc.dram_tensor("idx_asm", [128, NB], I32, kind="ExternalInput")
    eps_b_in = nc.dram_tensor("eps_b", [128, 2 * BSHC * D], F32,
                              kind="ExternalInput")
    ui_in = nc.dram_tensor("user_intent", [D, NINT], F32, kind="ExternalInput")
    ii_in = nc.dram_tensor("item_intent", [D, NINT], F32, kind="ExternalInput")
    lw_in = nc.dram_tensor("lin_w", [D, TS], F32, kind="ExternalInput")
    lb_in = nc.dram_tensor("lin_b_rep", [128, D], F32, kind="ExternalInput")

    partials = nc.dram_tensor("partials", [1, 16], F32, kind="ExternalOutput")

    yA = nc.dram_tensor("yA", [TROWS + 128, D], BF16, addr_space="Shared")
    yB = nc.dram_tensor("yB", [TROWS + 128, D], BF16, addr_space="Shared")
    shard_bf = nc.dram_tensor("shard_bf", [SPAD, D], BF16)
    accD = nc.dram_tensor("accD", [SPAD, D], F32)
    batchS = nc.dram_tensor("batchS", [SK * 128, D], F32)
    batchT = nc.dram_tensor("batchT", [NC * SK * 128, D], F32,
                            addr_space="Shared")
    uT_sh = nc.dram_tensor("uT_sh", [D, BSHC * 128], F32)
    iT_sh = nc.dram_tensor("iT_sh", [D, BSHC * 128], F32)
    uT_all = nc.dram_tensor("uT_all", [NC * D, BSHC * 128], F32,
                            addr_space="Shared")
    iT_all = nc.dram_tensor("iT_all", [NC * D, BSHC * 128], F32,
                            addr_space="Shared")

    groups = [list(range(NC))]

    with tile.TileContext(nc) as tc:
        with tc.tile_pool(name="const", bufs=1) as cp, \
             tc.tile_pool(name="work", bufs=2) as wp, \
             tc.tile_pool(name="spmm", bufs=48) as sp, \
             tc.tile_pool(name="curp", bufs=8) as curp, \
             tc.tile_pool(name="psum", bufs=1, space="PSUM") as pp:

            ident = cp.tile([128, 128], F32)
            make_identity(nc, ident[:])
            ones_col = cp.tile([128, 1], F32)
            nc.vector.memset(ones_col[:], 1.0)
            zrow_bf = cp.tile([1, D], BF16)
            nc.vector.memset(zrow_bf[:], 0.0)
            nc.sync.dma_start(out=yA[TROWS:TROWS + 1, :], in_=zrow_bf[:])
            nc.sync.dma_start(out=yB[TROWS:TROWS + 1, :], in_=zrow_bf[:])

            def load(shape, dt, src, name):
                t_ = cp.tile(shape, dt, tag=name)
                nc.sync.dma_start(out=t_[:], in_=src)
                return t_

            idxs = load([128, NI], I32, idx_spmm[:], "idxs")
            dinv = load([128, PC], F32, dinv_in[:], "dinv")
            dinv2 = load([128, PC], F32, dinv2_in[:], "dinv2")
            kmask = load([128, PC], F32, kmask_in[:], "kmask")
            idxo = load([128, SK], I32, idx_own[:], "idxo")
            idxa = load([128, NB], I32, idx_asm[:], "idxa")
            eps_b = load([128, 2 * BSHC * D], F32, eps_b_in[:], "epsb")
            lb_rep = load([128, D], F32, lb_in[:], "lb")
            ui_sb = load([D, NINT], F32, ui_in[:], "ui")
            ii_sb = load([D, NINT], F32, ii_in[:], "ii")
            lw_sb = load([D, TS], F32, lw_in[:], "lw")

            def transpose128(dst_ap, src_ap, P, Fr):
                ps = pp.tile([128, 128], F32, tag="tps")
                nc.tensor.transpose(out=ps[:Fr, :P], in_=src_ap,
                                    identity=ident[:P, :P])
                nc.vector.tensor_copy(dst_ap, ps[:Fr, :P])

            uiT = cp.tile([NINT, D], F32)
            transpose128(uiT[:], ui_sb[:], D, NINT)
            iiT = cp.tile([NINT, D], F32)
            transpose128(iiT[:], ii_sb[:], D, NINT)
            lwT = cp.tile([TS, D], F32)
            transpose128(lwT[:], lw_sb[:], D, TS)

            acc = cp.tile([128, PC * D], F32)
            nc.sync.dma_start(
                out=acc[:].rearrange("p (c d) -> p c d", d=D),
                in_=ego_perm[:].rearrange("(c p) d -> p c d", p=128))

            def dcol(tbl, tau):
                return tbl[:, tau:tau + 1].to_broadcast([128, D])

            def sbcol(tau):
                return shard_bf[:].rearrange(
                    "(c p) d -> p c d", p=128)[:, tau, :]

            def piece_collective(y_out, c0, c1, base):
                w = c1 - c0
                nc.gpsimd.collective_compute(
                    "AllGather", ALU.bypass, replica_groups=groups,
                    ins=[shard_bf[c0 * 128:c1 * 128, :]],
                    outs=[y_out[base:base + NC * w * 128, :]])

            # ---- y0 = d_inv * ego, piece-major with pipelined collectives
            base = 0
            for (c0, c1) in pieces:
                for tau in range(c0, c1):
                    yt = sp.tile([128, D], F32, tag="out_t")
                    nc.vector.tensor_tensor(
                        out=yt[:], in0=acc[:, tau * D:(tau + 1) * D],
                        in1=dcol(dinv, tau), op=ALU.mult)
                    yb = curp.tile([128, D], BF16, tag="ybf")
                    nc.vector.tensor_copy(yb[:], yt[:])
                    nc.sync.dma_start(out=sbcol(tau), in_=yb[:])
                piece_collective(yA, c0, c1, base)
                base += NC * (c1 - c0) * 128

            ybufs = [yA, yB]
            for layer in range(NL):
                y_in = ybufs[layer % 2]
                y_out = ybufs[(layer + 1) % 2]
                last = (layer == NL - 1)
                base = 0
                for pi, (c0, c1) in enumerate(pieces):
                    outs = {}
                    for tau in range(c0, c1):
                        out_t = sp.tile([128, D], F32, tag="out_t")
                        outs[tau] = out_t
                    sm = int(smax_u[c0:c1].max())
                    for s in range(sm):
                        for tau in range(c0, c1):
                            if s >= smax_u[tau]:
                                continue
                            icol = colof[(tau, s)]
                            kw = {} if s == 0 else dict(compute_op=ALU.add)
                            nc.gpsimd.indirect_dma_start(
                                out=outs[tau][:], out_offset=None,
                                in_=y_in[:],
                                in_offset=bass.IndirectOffsetOnAxis(
                                    ap=idxs[:, icol:icol + 1], axis=0),
                                **kw)
                    for tau in range(c0, c1):
                        out_t = outs[tau]
                        aslice = acc[:, tau * D:(tau + 1) * D]
                        cur = curp.tile([128, D], F32, tag="cur")
                        nc.vector.tensor_tensor(
                            out=cur[:], in0=out_t[:], in1=dcol(dinv, tau),
                            op=ALU.mult)
                        nc.vector.tensor_add(out=aslice, in0=aslice,
                                             in1=cur[:])
                        if not last:
                            nc.vector.tensor_tensor(
                                out=out_t[:], in0=out_t[:],
                                in1=dcol(dinv2, tau), op=ALU.mult)
                            yb = curp.tile([128, D], BF16, tag="ybf")
                            nc.vector.tensor_copy(yb[:], out_t[:])
                            nc.sync.dma_start(out=sbcol(tau), in_=yb[:])
                    if not last:
                        piece_collective(y_out, c0, c1, base)
                        base += NC * (c1 - c0) * 128

            # ---------------- downstream ----------------
            # write acc -> accD (cyclic layout), local batch gather,
            # AllGather compact buffers, assemble full-batch tables.
            nc.sync.dma_start(
                out=accD[:].rearrange("(c p) d -> p c d", p=128),
                in_=acc[:].rearrange("p (c d) -> p c d", d=D))

            own_reg = nc.alloc_register(mybir.EngineType.Pool, "ownreg")
            nc.gpsimd.reg_mov(own_reg, SPAD - 1)
            for q in range(SK):
                ot = curp.tile([128, D], F32, tag="own_t")
                nc.gpsimd.indirect_dma_start(
                    out=ot[:], out_offset=None, in_=accD[:],
                    in_offset=bass.IndirectOffsetOnAxis(
                        ap=idxo[:, q:q + 1], axis=0),
                    bounds_check=own_reg, oob_is_err=False)
                nc.sync.dma_start(out=batchS[q * 128:(q + 1) * 128, :],
                                  in_=ot[:])
            nc.gpsimd.collective_compute(
                "AllGather", ALU.bypass, replica_groups=groups,
                ins=[batchS[:]], outs=[batchT[:]])

            # assembled tables: IU | IP | NS  (ua_sh = IU[:,:BSHC*D] etc.)
            asm = cp.tile([128, NB * D], F32)
            for q in range(NB):
                nc.gpsimd.indirect_dma_start(
                    out=asm[:, q * D:(q + 1) * D], out_offset=None,
                    in_=batchT[:],
                    in_offset=bass.IndirectOffsetOnAxis(
                        ap=idxa[:, q:q + 1], axis=0))
            ua_full = asm[:, OFF_IU * D:(OFF_IU + BC) * D]
            ia_full = asm[:, OFF_IP * D:(OFF_IP + BC) * D]
            ua_sh = asm[:, OFF_IU * D:(OFF_IU + BSHC) * D]
            iap_sh = asm[:, OFF_IP * D:(OFF_IP + BSHC) * D]
            ian_sh = asm[:, OFF_NS * D:(OFF_NS + BSHC) * D]
            eps_u = eps_b[:, 0:BSHC * D]
            eps_p = eps_b[:, BSHC * D:2 * BSHC * D]

            def normalize_rows(x_ap, ncols):
                for q in range(ncols):
                    sl = x_ap[:, q * D:(q + 1) * D]
                    sq = wp.tile([128, D], F32, tag="sqj")
                    ss = wp.tile([128, 1], F32, tag="ssj")
                    nc.scalar.activation(sq[:], sl, ACTF.Square,
                                         accum_out=ss[:])
                    ls = wp.tile([128, 1], F32, tag="rsj")
                    nc.scalar.activation(ls[:], ss[:], ACTF.Ln)
                    rn = wp.tile([128, 1], F32, tag="rnj")
                    nc.scalar.activation(rn[:], ls[:], ACTF.Exp, scale=-0.5)
                    nc.vector.tensor_scalar_mul(sl, sl, rn[:])

            def intent_pipe(src_tl, w_sb, wT_sb, T_sh, T_all, tag):
                """Data-parallel intent: this core's BSHC chunks only.
                Returns normalized shard [128, BSHC*D]; full e2T arrives via
                AllGather into T_all."""
                sh_n = cp.tile([128, BSHC * D], F32, tag=f"in_{tag}")
                eT = cp.tile([D, BSHC * 128], F32, tag=f"iT_{tag}")
                for q in range(BSHC):
                    tl = src_tl[:, q * D:(q + 1) * D]
                    tT = wp.tile([D, 128], F32, tag="tT")
                    transpose128(tT[:], tl, 128, D)
                    zp = pp.tile([128, NINT], F32, tag="zp")
                    nc.tensor.matmul(out=zp[:], lhsT=tT[:], rhs=w_sb[:])
                    z = wp.tile([128, NINT], F32, tag="z")
                    nc.vector.tensor_copy(z[:], zp[:])
                    mx = wp.tile([128, 1], F32, tag="mx")
                    nc.vector.tensor_reduce(out=mx[:], in_=z[:], axis=AX.X,
                                            op=ALU.max)
                    nmx = wp.tile([128, 1], F32, tag="nmx")
                    nc.scalar.mul(nmx[:], mx[:], -1.0)
                    ex = wp.tile([128, NINT], F32, tag="ex")
                    se = wp.tile([128, 1], F32, tag="se")
                    nc.scalar.activation(ex[:], z[:], ACTF.Exp, bias=nmx[:],
                                         accum_out=se[:])
                    rse = wp.tile([128, 1], F32, tag="rse")
                    nc.vector.reciprocal(rse[:], se[:])
                    nc.vector.tensor_scalar_mul(ex[:], ex[:], rse[:])
                    exT = wp.tile([NINT, 128], F32, tag="exT")
                    transpose128(exT[:], ex[:], 128, NINT)
                    op_ = pp.tile([128, D], F32, tag="op")
                    nc.tensor.matmul(out=op_[:], lhsT=exT[:], rhs=wT_sb[:])
                    onrm = sh_n[:, q * D:(q + 1) * D]
                    nc.vector.tensor_copy(onrm, op_[:])
                normalize_rows(sh_n[:], BSHC)
                for q in range(BSHC):
                    transpose128(eT[:, q * 128:(q + 1) * 128],
                                 sh_n[:, q * D:(q + 1) * D], 128, D)
                nc.sync.dma_start(out=T_sh[:], in_=eT[:])
                nc.gpsimd.collective_compute(
                    "AllGather", ALU.bypass, replica_groups=groups,
                    ins=[T_sh[:]], outs=[T_all[:]])
                return sh_n

            u_i_n = intent_pipe(ua_full, ui_sb, uiT, uT_sh, uT_all, "u")
            i_i_n = intent_pipe(ia_full, ii_sb, iiT, iT_sh, iT_all, "i")

            # load gathered e2T: [NC*D, BSHC*128] -> NC tiles [D, BSHC*128]
            def load_e2T(T_all, tag):
                tiles = []
                for t_ in range(NC):
                    tl = cp.tile([D, BSHC * 128], F32, tag=f"{tag}{t_}")
                    nc.sync.dma_start(
                        out=tl[:], in_=T_all[t_ * D:(t_ + 1) * D, :])
                    tiles.append(tl)
                return tiles

            uT_tiles = load_e2T(uT_all, "uTt")
            iT_tiles = load_e2T(iT_all, "iTt")

            def gen_pipe(mean_tl, eps_tl, tag):
                gen_n = cp.tile([128, BSHC * D], F32, tag=f"gen_{tag}")
                genT = cp.tile([D, BSHC * 128], F32, tag=f"genT_{tag}")
                for q in range(BSHC):
                    msl = mean_tl[:, q * D:(q + 1) * D]
                    sp_t = wp.tile([128, TS], F32, tag="sp_t")
                    nc.scalar.activation(sp_t[:], msl[:, :TS], ACTF.Exp)
                    nc.vector.tensor_scalar_add(sp_t[:], sp_t[:], 1.0)
                    nc.scalar.activation(sp_t[:], sp_t[:], ACTF.Ln)
                    spT = wp.tile([TS, 128], F32, tag="spT")
                    transpose128(spT[:], sp_t[:], 128, TS)
                    stp = pp.tile([128, D], F32, tag="stp")
                    nc.tensor.matmul(out=stp[:], lhsT=spT[:], rhs=lwT[:])
                    std = wp.tile([128, D], F32, tag="std")
                    nc.vector.tensor_add(out=std[:], in0=stp[:],
                                         in1=lb_rep[:])
                    nc.vector.tensor_scalar_add(std[:], std[:], 1e-8)
                    g = gen_n[:, q * D:(q + 1) * D]
                    nc.vector.tensor_tensor(
                        out=g, in0=eps_tl[:, q * D:(q + 1) * D], in1=std[:],
                        op=ALU.mult)
                    nc.vector.tensor_add(out=g, in0=g, in1=msl)
                normalize_rows(gen_n[:], BSHC)
                for q in range(BSHC):
                    transpose128(genT[:, q * 128:(q + 1) * 128],
                                 gen_n[:, q * D:(q + 1) * D], 128, D)
                return gen_n, genT

            u_gen_n, u_gen_T = gen_pipe(ua_sh, eps_u, "gu")
            i_gen_n, i_gen_T = gen_pipe(iap_sh, eps_p, "gi")

            psb = cp.tile([1, 16], F32)
            nc.vector.memset(psb[:], 0.0)

            def part_sum(vec_ap, slot, P=128):
                ps = pp.tile([1, 1], F32, tag="pscal")
                nc.tensor.matmul(out=ps[:], lhsT=vec_ap, rhs=ones_col[:P, :])
                nc.vector.tensor_add(out=psb[:, slot:slot + 1],
                                     in0=psb[:, slot:slot + 1], in1=ps[:])

            def infonce(e1_n, e1_T, e2sh_n, e2_tiles, slot):
                lgs = wp.tile([128, BSHC], F32, tag="lgs")
                for q in range(BSHC):
                    prod = wp.tile([128, D], F32, tag="prod")
                    nc.vector.tensor_tensor(
                        out=prod[:], in0=e1_n[:, q * D:(q + 1) * D],
                        in1=e2sh_n[:, q * D:(q + 1) * D], op=ALU.mult)
                    pdot = wp.tile([128, 1], F32, tag="pdot")
                    nc.vector.tensor_reduce(out=pdot[:], in_=prod[:],
                                            axis=AX.X, op=ALU.add)
                    pex = wp.tile([128, 1], F32, tag="pex")
                    nc.scalar.activation(pex[:], pdot[:], ACTF.Exp,
                                         scale=1.0 / TEMP)
                    nss = wp.tile([128, BC], F32, tag="nss")
                    for ch in range(BC):
                        tl = e2_tiles[ch // BSHC]
                        pcol = (ch % BSHC) * 128
                        zp = pp.tile([128, 128], F32, tag="zneg")
                        nc.tensor.matmul(
                            out=zp[:], lhsT=e1_T[:, q * 128:(q + 1) * 128],
                            rhs=tl[:, pcol:pcol + 128])
                        ju = wp.tile([128, 128], F32, tag="ju")
                        nc.scalar.activation(
                            ju[:], zp[:], ACTF.Exp, scale=1.0 / TEMP,
                            accum_out=nss[:, ch:ch + 1])
                    nsum = wp.tile([128, 1], F32, tag="nsum")
                    nc.vector.tensor_reduce(out=nsum[:], in_=nss[:],
                                            axis=AX.X, op=ALU.add)
                    nc.vector.tensor_scalar_add(nsum[:], nsum[:], 1e-8)
                    rn = wp.tile([128, 1], F32, tag="rng")
                    nc.vector.reciprocal(rn[:], nsum[:])
                    qv = wp.tile([128, 1], F32, tag="qv")
                    nc.vector.tensor_tensor(out=qv[:], in0=pex[:], in1=rn[:],
                                            op=ALU.mult)
                    nc.vector.tensor_scalar_add(qv[:], qv[:], 1e-8)
                    nc.scalar.activation(lgs[:, q:q + 1], qv[:], ACTF.Ln)
                tot = wp.tile([128, 1], F32, tag="lgt")
                nc.vector.tensor_reduce(out=tot[:], in_=lgs[:], axis=AX.X,
                                        op=ALU.add)
                part_sum(tot[:], slot)

            infonce(u_gen_n, u_gen_T, u_i_n, uT_tiles, 2)
            infonce(i_gen_n, i_gen_T, i_i_n, iT_tiles, 3)

            # BPR
            dsc = wp.tile([128, BSHC], F32, tag="dsc")
            for q in range(BSHC):
                pr = wp.tile([128, D], F32, tag="bprp")
                nc.vector.tensor_tensor(
                    out=pr[:], in0=ua_sh[:, q * D:(q + 1) * D],
                    in1=iap_sh[:, q * D:(q + 1) * D], op=ALU.mult)
                ps_ = wp.tile([128, 1], F32, tag="bps")
                nc.vector.tensor_reduce(out=ps_[:], in_=pr[:], axis=AX.X,
                                        op=ALU.add)
                nr = wp.tile([128, D], F32, tag="bprn")
                nc.vector.tensor_tensor(
                    out=nr[:], in0=ua_sh[:, q * D:(q + 1) * D],
                    in1=ian_sh[:, q * D:(q + 1) * D], op=ALU.mult)
                ns_ = wp.tile([128, 1], F32, tag="bns")
                nc.vector.tensor_reduce(out=ns_[:], in_=nr[:], axis=AX.X,
                                        op=ALU.add)
                nc.vector.tensor_sub(out=dsc[:, q:q + 1], in0=ns_[:],
                                     in1=ps_[:])
            spl = wp.tile([128, BSHC], F32, tag="spl")
            nc.scalar.activation(spl[:], dsc[:], ACTF.Exp)
            nc.vector.tensor_scalar_add(spl[:], spl[:], 1.0)
            nc.scalar.activation(spl[:], spl[:], ACTF.Ln)
            bps = wp.tile([128, 1], F32, tag="bpst")
            nc.vector.tensor_reduce(out=bps[:], in_=spl[:], axis=AX.X,
                                    op=ALU.add)
            part_sum(bps[:], 0)

            # KL over own shard (from acc)
            KW = 8
            klcols = cp.tile([128, PC], F32)
            for g in range(math.ceil(PC / KW)):
                w0 = g * KW
                W = min(KW, PC - w0)
                mean_g = acc[:, w0 * D:(w0 + W) * D]
                spg = wp.tile([128, KW * TS], F32, tag="spg")
                nc.scalar.activation(
                    spg[:, :W * TS].rearrange("p (c d) -> p c d", d=TS),
                    acc[:, w0 * D:].rearrange(
                        "p (c d) -> p c d", d=D)[:, 0:W, 0:TS],
                    ACTF.Exp)
                nc.vector.tensor_scalar_add(spg[:, :W * TS], spg[:, :W * TS],
                                            1.0)
                nc.scalar.activation(spg[:, :W * TS], spg[:, :W * TS],
                                     ACTF.Ln)
                stdg = wp.tile([128, KW * D], F32, tag="stdg")
                for w in range(W):
                    spT = wp.tile([TS, 128], F32, tag="spTk")
                    transpose128(spT[:], spg[:, w * TS:(w + 1) * TS], 128, TS)
                    stp = pp.tile([128, D], F32, tag="stp")
                    nc.tensor.matmul(out=stp[:], lhsT=spT[:], rhs=lwT[:])
                    sw = stdg[:, w * D:(w + 1) * D]
                    nc.vector.tensor_add(out=sw, in0=stp[:], in1=lb_rep[:])
                    nc.vector.tensor_scalar_add(sw, sw, 1e-8)
                m2 = wp.tile([128, KW * D], F32, tag="m2")
                nc.scalar.activation(m2[:, :W * D], mean_g, ACTF.Square)
                exg = wp.tile([128, KW * D], F32, tag="exg")
                nc.scalar.activation(exg[:, :W * D], stdg[:, :W * D],
                                     ACTF.Exp, scale=2.0)
                t1 = wp.tile([128, KW * D], F32, tag="t1")
                nc.scalar.activation(t1[:, :W * D], stdg[:, :W * D],
                                     ACTF.Copy, bias=0.0, scale=2.0)
                nc.vector.tensor_scalar_add(t1[:, :W * D], t1[:, :W * D], 1.0)
                nc.vector.tensor_sub(out=t1[:, :W * D], in0=t1[:, :W * D],
                                     in1=m2[:, :W * D])
                nc.vector.tensor_sub(out=t1[:, :W * D], in0=t1[:, :W * D],
                                     in1=exg[:, :W * D])
                nc.vector.tensor_reduce(
                    out=klcols[:, w0:w0 + W],
                    in_=t1[:, :W * D].rearrange("p (c d) -> p c d", d=D),
                    axis=AX.X, op=ALU.add)
            nc.vector.tensor_tensor(out=klcols[:], in0=klcols[:],
                                    in1=kmask[:], op=ALU.mult)
            ktot = wp.tile([128, 1], F32, tag="ktot")
            nc.vector.tensor_reduce(out=ktot[:], in_=klcols[:], axis=AX.X,
                                    op=ALU.add)
            part_sum(ktot[:], 1)

            nc.sync.dma_start(out=partials[:], in_=psb[:])

    return nc


# --------------------------------------------------------------------------
# entry
# --------------------------------------------------------------------------

def prepare(inputs, c, for_sim=False):
    """Returns (nc, in_maps)."""
    NC, D, SPAD = c["NC"], c["D"], c["SPAD"]
    per_core, perm_pos, trow_vec, ZROW = host_prep(inputs, c)

    users0 = np.asarray(inputs["users"]).astype(np.int64)
    pos0 = np.asarray(inputs["pos_items"]).astype(np.int64)
    neg0 = np.asarray(inputs["neg_items"]).astype(np.int64)
    N_USERS, B, BSH, BC, BSHC = (c["N_USERS"], c["B"], c["BSH"], c["BCOLS"],
                                 c["BSHC"])
    SENT = 1 << 20

    # ---- 2-phase batch tables ----
    # global batch node list: users | pos | neg (original order)
    nodes = np.concatenate([users0, N_USERS + pos0, N_USERS + neg0])
    gpos = perm_pos[nodes]                      # perm position (k*SPAD+j)
    owner = gpos // SPAD
    localj = gpos % SPAD
    # local cyclic row in accD: row = j (accD is [SPAD, D] cyclic c*128+p ->
    # row j directly since accD[(c p)] with j = c*128+p)
    # compact position assignment per core, in global slot order
    NBATCH = len(nodes)
    own_lists = [np.where(owner == k)[0] for k in range(NC)]
    max_own = max(len(x) for x in own_lists)
    SK = math.ceil(max_own / 128)
    c["SK"] = SK
    # compact position of global slot i on its owner core
    compact_pos = np.empty(NBATCH, dtype=np.int64)
    idx_own_cores = []
    for k in range(NC):
        lst = own_lists[k]
        compact_pos[lst] = np.arange(len(lst))
        col = np.full(SK * 128, SENT, dtype=np.int64)
        col[:len(lst)] = localj[lst]
        # gather op q covers positions q*128..q*128+127; position i ->
        # partition i%128 in the [128, D] out tile
        idx_own_cores.append(col.reshape(SK, 128).T.astype(np.int32))
    # assembly index: batchT row of global slot i = owner*SK*128 + compact
    asm_row = owner * (SK * 128) + compact_pos

    nc_prog = None  # built after SK known

    def cycb_rows(rows):
        m = len(rows) // 128
        return rows.reshape(m, 128).T.astype(np.int32)

    eps_np = np.asarray(inputs["eps"], dtype=np.float32)
    ui_np = np.asarray(inputs["user_intent"], dtype=np.float32)
    ii_np = np.asarray(inputs["item_intent"], dtype=np.float32)
    lw_np = np.asarray(inputs["lin_w"], dtype=np.float32)
    lb_rep = np.tile(np.asarray(inputs["lin_b"],
                                dtype=np.float32)[None, :], (128, 1))

    in_maps = []
    for k in range(NC):
        rot = np.roll(np.arange(B), -k * BSH)
        # assembled columns: IU (users rotated, full B) | IP (pos rotated,
        # full B) | NS (neg rotated, shard only)
        iu_rows = asm_row[rot]                       # users
        ip_rows = asm_row[B + rot]                   # pos
        ns_rows = asm_row[2 * B + rot[:BSH]]         # neg shard
        idx_asm = np.concatenate([
            cycb_rows(iu_rows), cycb_rows(ip_rows), cycb_rows(ns_rows),
        ], axis=1)
        # eps pre-gather: users shard + pos shard (original node indices)
        eu = eps_np[users0[rot][:BSH]]               # [BSH, D]
        ep = eps_np[N_USERS + pos0[rot][:BSH]]
        def cycd(x):
            return x.reshape(BSHC, 128, D).transpose(1, 0, 2).reshape(
                128, BSHC * D)
        eps_b = np.concatenate([cycd(eu), cycd(ep)], axis=1)
        pk = per_core[k]
        in_maps.append(dict(
            ego_perm=pk["ego_perm"], idx_spmm=pk["idx_spmm"],
            dinv=pk["dinv"], dinv2=pk["dinv2"], kmask=pk["kmask"],
            idx_own=idx_own_cores[k], idx_asm=idx_asm, eps_b=eps_b,
            user_intent=ui_np, item_intent=ii_np, lin_w=lw_np,
            lin_b_rep=lb_rep))

    nc_prog = build_bass(c)
    if not for_sim:
        split_multi_waits(nc_prog)
    return nc_prog, in_maps


def combine(results, c, inputs):
    NC, B, N = c["NC"], c["B"], c["N"]
    P = np.stack([np.asarray(results[k]["partials"][0], dtype=np.float64)
                  for k in range(NC)])
    bpr = P[:, 0].sum() / B
    kl = c["KL_REG"] * (-0.5 * P[:, 1].sum()) / N
    gen_loss = np.float32(bpr + kl)
    cl_loss = np.float32(c["SSL_REG"] * (-(P[:, 2].sum()) - P[:, 3].sum()) / B)

    users = np.asarray(inputs["users"]).astype(np.int64)
    pos = np.asarray(inputs["pos_items"]).astype(np.int64)
    neg = np.asarray(inputs["neg_items"]).astype(np.int64)
    ue = np.asarray(inputs["user_emb"], dtype=np.float64)
    ie = np.asarray(inputs["item_emb"], dtype=np.float64)
    ui = np.asarray(inputs["user_intent"], dtype=np.float64)
    ii = np.asarray(inputs["item_intent"], dtype=np.float64)
    emb_loss = np.float32(c["EMB_REG"] * (
        np.sum(np.square(ue[users])) + np.sum(np.square(ie[pos]))
        + np.sum(np.square(ie[neg]))))
    int_loss = np.float32(c["INT_REG"] * (np.sum(np.square(ui))
                                          + np.sum(np.square(ii))))
    return (gen_loss, cl_loss, emb_loss, int_loss)


def kernel(**inputs):
    c = derive(default_cfg())
    nc, in_maps = prepare(inputs, c)
    res = run_bass_kernel_spmd(nc, in_maps, list(range(c["NC"])))
    return combine(res.results, c, inputs)


# revision 9
# speedup vs baseline: 1.0233x; 1.0233x over previous
"""DVGCL (GNN message passing + contrastive losses) on 8 Trainium2 cores.

v2. Sharding: node dim N split 8 ways by destination; each shard degree-sorted
and laid out cyclically (pos j -> partition j%128, col j//128); the permutation
is folded into every index array on the host. Propagation gathers the
pre-scaled bf16 table y = d_inv * cur via per-(tile,slot) indirect DMAs with
CCE-add accumulation (one 128-row gather per op is the HW limit; ~1.2us/op of
Q7 descriptor-gen is the floor). Sentinel slots point at a dedicated zero row
of the table, so no memsets or bounds checks are needed.

Layer pipelining: each layer's y table is AllGathered in NPIECE column-range
pieces; gathers are issued piece-major so piece p's scale+write+collective
overlaps piece p+1's gathers, leaving only the last piece's collective on the
critical path at each layer boundary.

The final 38.6MB all_emb AllGather is replaced by a 2-phase batch path: each
core indirect-gathers the ~B*3/8 batch rows it owns from its local shard into
a compact buffer, one small AllGather shares those, and 68 assembly gathers
rebuild the full-batch tables per core. Intent pipes are data-parallel (each
core computes its 1/8 of the batch; normalized transposed slices are
AllGathered for the InfoNCE negatives). eps rows are pre-gathered on the host;
the emb/int regularizer losses are pure functions of the inputs and are
computed on the host in combine().

Walrus codegen accepts at most ONE sync wait per instruction, so
split_multi_waits hoists extras onto same-engine NoOps after Tile scheduling.
"""
import math
import numpy as np

import concourse.bass as bass
import concourse.mybir as mybir
import concourse.tile as tile
from concourse.bass_utils import run_bass_kernel_spmd
from concourse.masks import make_identity

F32 = mybir.dt.float32
BF16 = mybir.dt.bfloat16
I32 = mybir.dt.int32
AX = mybir.AxisListType
ALU = mybir.AluOpType
ACTF = mybir.ActivationFunctionType


def default_cfg():
    return dict(
        N_USERS=50000, N_ITEMS=100000, D=64, N_LAYERS=3, N_INTENTS=128,
        T_SIZE=32, TEMP=0.2, KL_REG=0.01, EMB_REG=1e-5, INT_REG=1e-5,
        SSL_REG=0.1, B=4096, NC=8, NPIECE=8,
    )


def derive(cfg):
    c = dict(cfg)
    c["N"] = c["N_USERS"] + c["N_ITEMS"]
    assert c["N"] % c["NC"] == 0
    c["SHARD"] = c["N"] // c["NC"]
    c["PC"] = math.ceil(c["SHARD"] / 128)
    c["SPAD"] = 128 * c["PC"]
    c["TROWS"] = c["NC"] * c["SPAD"]
    assert c["B"] % 128 == 0 and (c["B"] // c["NC"]) % 128 == 0
    c["BCOLS"] = c["B"] // 128
    c["BSH"] = c["B"] // c["NC"]
    c["BSHC"] = c["BSH"] // 128
    # piece column ranges (NPIECE ranges over PC columns)
    w = math.ceil(c["PC"] / c["NPIECE"])
    edges = [min(p * w, c["PC"]) for p in range(c["NPIECE"] + 1)]
    c["PIECES"] = [(edges[p], edges[p + 1]) for p in range(c["NPIECE"])
                   if edges[p + 1] > edges[p]]
    return c


# --------------------------------------------------------------------------
# wait splitting post-pass (walrus: max 1 sync wait per instruction)
# --------------------------------------------------------------------------

def split_multi_waits(nc, max_waits=1):
    n = 0
    for f in nc.m.functions:
        for b in f.blocks:
            insts = b.instructions
            items = list(insts)
            out = []
            for i in items:
                si = i.sync_info
                w = list(si.on_wait) if si and si.on_wait else []
                if len(w) > max_waits:
                    for x in w[:-max_waits]:
                        n += 1
                        out.append(mybir.InstNoOp(
                            name=f"waitsplit-{n}",
                            sync_info=mybir.SyncInfo(on_wait=[x], on_update=[]),
                            engine=i.engine, bass_nofuse=True))
                    si.on_wait = w[-max_waits:]
                out.append(i)
            insts.clear()
            insts.extend(out)
    return n


# --------------------------------------------------------------------------
# host prep
# --------------------------------------------------------------------------

def host_prep(inputs, c):
    N, NC, SHARD, SPAD, PC, D = (c["N"], c["NC"], c["SHARD"], c["SPAD"],
                                 c["PC"], c["D"])
    h = np.asarray(inputs["h_list"]).astype(np.int64)
    t = np.asarray(inputs["t_list"]).astype(np.int64)

    deg = np.bincount(h, minlength=N).astype(np.int64)
    with np.errstate(divide="ignore"):
        d_inv = (deg.astype(np.float64) ** -0.5).astype(np.float32)

    # perm position j of node n: shard k = n // SHARD, degree-sorted inside
    perm_pos = np.empty(N, dtype=np.int64)   # node -> (core, j)
    inv_order = []
    for k in range(NC):
        lo = k * SHARD
        order = np.argsort(deg[lo:lo + SHARD], kind="stable")
        perm_pos[lo + order] = k * SPAD + np.arange(SHARD)
        inv_order.append(lo + order)

    # piece-major AllGather table layout: piece p holds cols [c0, c1) of all
    # cores, core-major inside the piece. trow(k, j):
    pieces = c["PIECES"]
    piece_of_col = np.empty(PC, dtype=np.int64)
    piece_base = np.empty(PC, dtype=np.int64)   # table row base of col's piece
    col_in_piece = np.empty(PC, dtype=np.int64)
    base = 0
    for (c0, c1) in pieces:
        piece_of_col[c0:c1] = base
        for cc in range(c0, c1):
            piece_base[cc] = base
            col_in_piece[cc] = cc - c0
        base += NC * (c1 - c0) * 128
    TROWS_TBL = base  # == NC * SPAD

    def trow_of_pos(pos):
        """global perm position (k*SPAD + j) -> table row in piece-major"""
        k = pos // SPAD
        j = pos % SPAD
        p128 = j % 128
        cc = j // 128
        (w0,) = (piece_of_col[cc],)
        c0 = cc - col_in_piece[cc]
        # width of the piece this col is in
        w = None
        for (a, b) in pieces:
            if a <= cc < b:
                w = b - a
                break
        return (piece_base[cc] + k * w * 128 + (cc - c0) * 128 + p128)

    # vectorized trow
    piece_w = np.empty(PC, dtype=np.int64)
    piece_c0 = np.empty(PC, dtype=np.int64)
    for (a, b) in pieces:
        piece_w[a:b] = b - a
        piece_c0[a:b] = a

    def trow_vec(pos):
        pos = np.asarray(pos, dtype=np.int64)
        k = pos // SPAD
        j = pos % SPAD
        p128 = j % 128
        cc = j // 128
        return (piece_base[cc] + k * piece_w[cc] * 128
                + (cc - piece_c0[cc]) * 128 + p128)

    ZROW = TROWS_TBL  # dedicated zero row

    dest_pos = perm_pos[h]
    eorder = np.argsort(dest_pos, kind="stable")
    dpos_s = dest_pos[eorder]
    src_rows = trow_vec(perm_pos[t[eorder]])

    ego = np.concatenate([
        np.asarray(inputs["user_emb"], dtype=np.float32),
        np.asarray(inputs["item_emb"], dtype=np.float32),
    ], axis=0)

    # per-core per-tile slot columns (values = table rows; pad = ZROW)
    core_cols = []       # list of dict[(tau, s)] -> int64[128]
    core_smax = []
    for k in range(NC):
        basek = k * SPAD
        lo_i = np.searchsorted(dpos_s, basek)
        hi_i = np.searchsorted(dpos_s, basek + SHARD)
        dj = dpos_s[lo_i:hi_i] - basek
        sj = src_rows[lo_i:hi_i]
        degl = np.zeros(SPAD, dtype=np.int64)
        np.add.at(degl, dj, 1)
        starts = np.zeros(SPAD + 1, dtype=np.int64)
        np.cumsum(degl, out=starts[1:])
        cols = {}
        smax = np.zeros(PC, dtype=np.int64)
        for tau in range(PC):
            jlo = tau * 128
            dtile = degl[jlo:jlo + 128]
            smax[tau] = int(dtile.max())
            for s in range(smax[tau]):
                col = np.full(128, ZROW, dtype=np.int64)
                sel = dtile > s
                col[sel] = sj[starts[jlo:jlo + 128][sel] + s]
                cols[(tau, s)] = col
        core_cols.append(cols)
        core_smax.append(smax)

    # SPMD union plan: per tile, slot count = max across cores (>=1 so the
    # first op's unconditional write covers pad lanes with zeros)
    smax_u = np.maximum(np.max(np.stack(core_smax), axis=0), 1)
    NI = int(smax_u.sum())
    c["SMAX_U"] = smax_u
    c["NI"] = NI

    # idx_spmm layout: piece-major, within piece slot-major round-robin over
    # that piece's tiles. colof[(tau, s)] -> column index in idx_spmm.
    colof = {}
    _ic = 0
    order_ops = []
    for (c0, c1) in c["PIECES"]:
        sm = int(smax_u[c0:c1].max())
        for s in range(sm):
            for tau in range(c0, c1):
                if s < smax_u[tau]:
                    colof[(tau, s)] = _ic
                    order_ops.append((tau, s))
                    _ic += 1
    assert _ic == NI
    c["COLOF"] = colof
    c["OP_ORDER"] = order_ops

    per_core = []
    for k in range(NC):
        idx = np.full((128, NI), ZROW, dtype=np.int32)
        for (tau, s), col in colof.items():
            v = core_cols[k].get((tau, s))
            if v is not None:
                idx[:, col] = v
        def cyc(vec):
            return vec.reshape(PC, 128).T.copy()
        dloc = np.zeros(SPAD, dtype=np.float32)
        dloc[:SHARD] = d_inv[inv_order[k]]
        mask = np.zeros(SPAD, dtype=np.float32)
        mask[:SHARD] = 1.0
        egp = np.zeros((SPAD, D), dtype=np.float32)
        egp[:SHARD] = ego[inv_order[k]]
        per_core.append(dict(
            idx_spmm=idx, dinv=cyc(dloc), dinv2=cyc(dloc * dloc),
            kmask=cyc(mask), ego_perm=egp))

    return per_core, perm_pos, trow_vec, ZROW


# --------------------------------------------------------------------------
# device program
# --------------------------------------------------------------------------

def build_bass(c):
    NC, D, PC, SPAD, TROWS = c["NC"], c["D"], c["PC"], c["SPAD"], c["TROWS"]
    BC, BSHC, NI, SK = c["BCOLS"], c["BSHC"], c["NI"], c["SK"]
    NINT, TS, NL = c["N_INTENTS"], c["T_SIZE"], c["N_LAYERS"]
    TEMP = c["TEMP"]
    smax_u = c["SMAX_U"]
    colof = c["COLOF"]
    pieces = c["PIECES"]
    # assembled batch tile columns: IU (BC) | IP (BC) | NS (BSHC)
    NB = 2 * BC + BSHC
    OFF_IU, OFF_IP, OFF_NS = 0, BC, 2 * BC

    nc = bass.Bass(num_devices=NC)

    ego_perm = nc.dram_tensor("ego_perm", [SPAD, D], F32, kind="ExternalInput")
    idx_spmm = nc.dram_tensor("idx_spmm", [128, NI], I32, kind="ExternalInput")
    dinv_in = nc.dram_tensor("dinv", [128, PC], F32, kind="ExternalInput")
    dinv2_in = nc.dram_tensor("dinv2", [128, PC], F32, kind="ExternalInput")
    kmask_in = nc.dram_tensor("kmask", [128, PC], F32, kind="ExternalInput")
    idx_own = nc.dram_tensor("idx_own", [128, SK], I32, kind="ExternalInput")
    idx_asm = nc.dram_tensor("idx_asm", [128, NB], I32, kind="ExternalInput")
    eps_b_in = nc.dram_tensor("eps_b", [128, 2 * BSHC * D], F32,
                              kind="ExternalInput")
    ui_in = nc.dram_tensor("user_intent", [D, NINT], F32, kind="ExternalInput")
    ii_in = nc.dram_tensor("item_intent", [D, NINT], F32, kind="ExternalInput")
    lw_in = nc.dram_tensor("lin_w", [D, TS], F32, kind="ExternalInput")
    lb_in = nc.dram_tensor("lin_b_rep", [128, D], F32, kind="ExternalInput")

    partials = nc.dram_tensor("partials", [1, 16], F32, kind="ExternalOutput")

    yA = nc.dram_tensor("yA", [TROWS + 128, D], BF16, addr_space="Shared")
    yB = nc.dram_tensor("yB", [TROWS + 128, D], BF16, addr_space="Shared")
    shard_bf = nc.dram_tensor("shard_bf", [SPAD, D], BF16)
    accD = nc.dram_tensor("accD", [SPAD, D], F32)
    batchS = nc.dram_tensor("batchS", [SK * 128, D], F32)
    batchT = nc.dram_tensor("batchT", [NC * SK * 128, D], F32,
                            addr_space="Shared")
    uT_sh = nc.dram_tensor("uT_sh", [D, BSHC * 128], F32)
    iT_sh = nc.dram_tensor("iT_sh", [D, BSHC * 128], F32)
    uT_all = nc.dram_tensor("uT_all", [NC * D, BSHC * 128], F32,
                            addr_space="Shared")
    iT_all = nc.dram_tensor("iT_all", [NC * D, BSHC * 128], F32,
                            addr_space="Shared")

    groups = [list(range(NC))]

    with tile.TileContext(nc) as tc:
        with tc.tile_pool(name="const", bufs=1) as cp, \
             tc.tile_pool(name="work", bufs=2) as wp, \
             tc.tile_pool(name="spmm", bufs=48) as sp, \
             tc.tile_pool(name="curp", bufs=8) as curp, \
             tc.tile_pool(name="psum", bufs=1, space="PSUM") as pp:

            ident = cp.tile([128, 128], F32)
            make_identity(nc, ident[:])
            ones_col = cp.tile([128, 1], F32)
            nc.vector.memset(ones_col[:], 1.0)
            zrow_bf = cp.tile([1, D], BF16)
            nc.vector.memset(zrow_bf[:], 0.0)
            nc.sync.dma_start(out=yA[TROWS:TROWS + 1, :], in_=zrow_bf[:])
            nc.sync.dma_start(out=yB[TROWS:TROWS + 1, :], in_=zrow_bf[:])

            def load(shape, dt, src, name):
                t_ = cp.tile(shape, dt, tag=name)
                nc.sync.dma_start(out=t_[:], in_=src)
                return t_

            idxs = load([128, NI], I32, idx_spmm[:], "idxs")
            dinv = load([128, PC], F32, dinv_in[:], "dinv")
            dinv2 = load([128, PC], F32, dinv2_in[:], "dinv2")
            kmask = load([128, PC], F32, kmask_in[:], "kmask")
            idxo = load([128, SK], I32, idx_own[:], "idxo")
            idxa = load([128, NB], I32, idx_asm[:], "idxa")
            eps_b = load([128, 2 * BSHC * D], F32, eps_b_in[:], "epsb")
            lb_rep = load([128, D], F32, lb_in[:], "lb")
            ui_sb = load([D, NINT], F32, ui_in[:], "ui")
            ii_sb = load([D, NINT], F32, ii_in[:], "ii")
            lw_sb = load([D, TS], F32, lw_in[:], "lw")

            def transpose128(dst_ap, src_ap, P, Fr):
                ps = pp.tile([128, 128], F32, tag="tps")
                nc.tensor.transpose(out=ps[:Fr, :P], in_=src_ap,
                                    identity=ident[:P, :P])
                nc.vector.tensor_copy(dst_ap, ps[:Fr, :P])

            uiT = cp.tile([NINT, D], F32)
            transpose128(uiT[:], ui_sb[:], D, NINT)
            iiT = cp.tile([NINT, D], F32)
            transpose128(iiT[:], ii_sb[:], D, NINT)
            lwT = cp.tile([TS, D], F32)
            transpose128(lwT[:], lw_sb[:], D, TS)

            acc = cp.tile([128, PC * D], F32)
            nc.sync.dma_start(
                out=acc[:].rearrange("p (c d) -> p c d", d=D),
                in_=ego_perm[:].rearrange("(c p) d -> p c d", p=128))

            def dcol(tbl, tau):
                return tbl[:, tau:tau + 1].to_broadcast([128, D])

            def sbcol(tau):
                return shard_bf[:].rearrange(
                    "(c p) d -> p c d", p=128)[:, tau, :]

            def piece_collective(y_out, c0, c1, base):
                w = c1 - c0
                nc.gpsimd.collective_compute(
                    "AllGather", ALU.bypass, replica_groups=groups,
                    ins=[shard_bf[c0 * 128:c1 * 128, :]],
                    outs=[y_out[base:base + NC * w * 128, :]])

            # ---- y0 = d_inv * ego, piece-major with pipelined collectives
            base = 0
            for (c0, c1) in pieces:
                for tau in range(c0, c1):
                    yt = sp.tile([128, D], F32, tag="out_t")
                    nc.vector.tensor_tensor(
                        out=yt[:], in0=acc[:, tau * D:(tau + 1) * D],
                        in1=dcol(dinv, tau), op=ALU.mult)
                    yb = curp.tile([128, D], BF16, tag="ybf")
                    nc.vector.tensor_copy(yb[:], yt[:])
                    nc.sync.dma_start(out=sbcol(tau), in_=yb[:])
                piece_collective(yA, c0, c1, base)
                base += NC * (c1 - c0) * 128
            # (y0 collectives overlap the ego-phase; no gathers to hide
            # behind yet, so keep them immediate)

            ybufs = [yA, yB]
            for layer in range(NL):
                y_in = ybufs[layer % 2]
                y_out = ybufs[(layer + 1) % 2]
                last = (layer == NL - 1)
                base = 0
                pending = []   # (c0, c1, base) awaiting collective issue
                for pi, (c0, c1) in enumerate(pieces):
                    outs = {}
                    for tau in range(c0, c1):
                        out_t = sp.tile([128, D], F32, tag="out_t")
                        outs[tau] = out_t
                    sm = int(smax_u[c0:c1].max())
                    for s in range(sm):
                        for tau in range(c0, c1):
                            if s >= smax_u[tau]:
                                continue
                            icol = colof[(tau, s)]
                            kw = {} if s == 0 else dict(compute_op=ALU.add)
                            nc.gpsimd.indirect_dma_start(
                                out=outs[tau][:], out_offset=None,
                                in_=y_in[:],
                                in_offset=bass.IndirectOffsetOnAxis(
                                    ap=idxs[:, icol:icol + 1], axis=0),
                                **kw)
                    if not last and pending:
                        piece_collective(y_out, *pending.pop(0))
                    for tau in range(c0, c1):
                        out_t = outs[tau]
                        aslice = acc[:, tau * D:(tau + 1) * D]
                        cur = curp.tile([128, D], F32, tag="cur")
                        nc.vector.tensor_tensor(
                            out=cur[:], in0=out_t[:], in1=dcol(dinv, tau),
                            op=ALU.mult)
                        nc.vector.tensor_add(out=aslice, in0=aslice,
                                             in1=cur[:])
                        if not last:
                            nc.vector.tensor_tensor(
                                out=out_t[:], in0=out_t[:],
                                in1=dcol(dinv2, tau), op=ALU.mult)
                            yb = curp.tile([128, D], BF16, tag="ybf")
                            nc.vector.tensor_copy(yb[:], out_t[:])
                            nc.sync.dma_start(out=sbcol(tau), in_=yb[:])
                    if not last:
                        pending.append((c0, c1, base))
                        base += NC * (c1 - c0) * 128
                if not last:
                    for args in pending:
                        piece_collective(y_out, *args)

            # ---------------- downstream ----------------
            # write acc -> accD (cyclic layout), local batch gather,
            # AllGather compact buffers, assemble full-batch tables.
            nc.sync.dma_start(
                out=accD[:].rearrange("(c p) d -> p c d", p=128),
                in_=acc[:].rearrange("p (c d) -> p c d", d=D))

            own_reg = nc.alloc_register(mybir.EngineType.Pool, "ownreg")
            nc.gpsimd.reg_mov(own_reg, SPAD - 1)
            for q in range(SK):
                ot = curp.tile([128, D], F32, tag="own_t")
                nc.gpsimd.indirect_dma_start(
                    out=ot[:], out_offset=None, in_=accD[:],
                    in_offset=bass.IndirectOffsetOnAxis(
                        ap=idxo[:, q:q + 1], axis=0),
                    bounds_check=own_reg, oob_is_err=False)
                nc.sync.dma_start(out=batchS[q * 128:(q + 1) * 128, :],
                                  in_=ot[:])
            nc.gpsimd.collective_compute(
                "AllGather", ALU.bypass, replica_groups=groups,
                ins=[batchS[:]], outs=[batchT[:]])

            # assembled tables: IU | IP | NS  (ua_sh = IU[:,:BSHC*D] etc.)
            asm = cp.tile([128, NB * D], F32)
            for q in range(NB):
                nc.gpsimd.indirect_dma_start(
                    out=asm[:, q * D:(q + 1) * D], out_offset=None,
                    in_=batchT[:],
                    in_offset=bass.IndirectOffsetOnAxis(
                        ap=idxa[:, q:q + 1], axis=0))
            ua_full = asm[:, OFF_IU * D:(OFF_IU + BC) * D]
            ia_full = asm[:, OFF_IP * D:(OFF_IP + BC) * D]
            ua_sh = asm[:, OFF_IU * D:(OFF_IU + BSHC) * D]
            iap_sh = asm[:, OFF_IP * D:(OFF_IP + BSHC) * D]
            ian_sh = asm[:, OFF_NS * D:(OFF_NS + BSHC) * D]
            eps_u = eps_b[:, 0:BSHC * D]
            eps_p = eps_b[:, BSHC * D:2 * BSHC * D]

            def normalize_rows(x_ap, ncols):
                for q in range(ncols):
                    sl = x_ap[:, q * D:(q + 1) * D]
                    sq = wp.tile([128, D], F32, tag="sqj")
                    ss = wp.tile([128, 1], F32, tag="ssj")
                    nc.scalar.activation(sq[:], sl, ACTF.Square,
                                         accum_out=ss[:])
                    ls = wp.tile([128, 1], F32, tag="rsj")
                    nc.scalar.activation(ls[:], ss[:], ACTF.Ln)
                    rn = wp.tile([128, 1], F32, tag="rnj")
                    nc.scalar.activation(rn[:], ls[:], ACTF.Exp, scale=-0.5)
                    nc.vector.tensor_scalar_mul(sl, sl, rn[:])

            def intent_pipe(src_tl, w_sb, wT_sb, T_sh, T_all, tag):
                """Data-parallel intent: this core's BSHC chunks only.
                Returns normalized shard [128, BSHC*D]; full e2T arrives via
                AllGather into T_all."""
                sh_n = cp.tile([128, BSHC * D], F32, tag=f"in_{tag}")
                eT = cp.tile([D, BSHC * 128], F32, tag=f"iT_{tag}")
                for q in range(BSHC):
                    tl = src_tl[:, q * D:(q + 1) * D]
                    tT = wp.tile([D, 128], F32, tag="tT")
                    transpose128(tT[:], tl, 128, D)
                    zp = pp.tile([128, NINT], F32, tag="zp")
                    nc.tensor.matmul(out=zp[:], lhsT=tT[:], rhs=w_sb[:])
                    z = wp.tile([128, NINT], F32, tag="z")
                    nc.vector.tensor_copy(z[:], zp[:])
                    mx = wp.tile([128, 1], F32, tag="mx")
                    nc.vector.tensor_reduce(out=mx[:], in_=z[:], axis=AX.X,
                                            op=ALU.max)
                    nmx = wp.tile([128, 1], F32, tag="nmx")
                    nc.scalar.mul(nmx[:], mx[:], -1.0)
                    ex = wp.tile([128, NINT], F32, tag="ex")
                    se = wp.tile([128, 1], F32, tag="se")
                    nc.scalar.activation(ex[:], z[:], ACTF.Exp, bias=nmx[:],
                                         accum_out=se[:])
                    rse = wp.tile([128, 1], F32, tag="rse")
                    nc.vector.reciprocal(rse[:], se[:])
                    nc.vector.tensor_scalar_mul(ex[:], ex[:], rse[:])
                    exT = wp.tile([NINT, 128], F32, tag="exT")
                    transpose128(exT[:], ex[:], 128, NINT)
                    op_ = pp.tile([128, D], F32, tag="op")
                    nc.tensor.matmul(out=op_[:], lhsT=exT[:], rhs=wT_sb[:])
                    onrm = sh_n[:, q * D:(q + 1) * D]
                    nc.vector.tensor_copy(onrm, op_[:])
                normalize_rows(sh_n[:], BSHC)
                for q in range(BSHC):
                    transpose128(eT[:, q * 128:(q + 1) * 128],
                                 sh_n[:, q * D:(q + 1) * D], 128, D)
                nc.sync.dma_start(out=T_sh[:], in_=eT[:])
                nc.gpsimd.collective_compute(
                    "AllGather", ALU.bypass, replica_groups=groups,
                    ins=[T_sh[:]], outs=[T_all[:]])
                return sh_n

            u_i_n = intent_pipe(ua_full, ui_sb, uiT, uT_sh, uT_all, "u")
            i_i_n = intent_pipe(ia_full, ii_sb, iiT, iT_sh, iT_all, "i")

            # load gathered e2T: [NC*D, BSHC*128] -> NC tiles [D, BSHC*128]
            def load_e2T(T_all, tag):
                tiles = []
                for t_ in range(NC):
                    tl = cp.tile([D, BSHC * 128], F32, tag=f"{tag}{t_}")
                    nc.sync.dma_start(
                        out=tl[:], in_=T_all[t_ * D:(t_ + 1) * D, :])
                    tiles.append(tl)
                return tiles

            uT_tiles = load_e2T(uT_all, "uTt")
            iT_tiles = load_e2T(iT_all, "iTt")

            def gen_pipe(mean_tl, eps_tl, tag):
                gen_n = cp.tile([128, BSHC * D], F32, tag=f"gen_{tag}")
                genT = cp.tile([D, BSHC * 128], F32, tag=f"genT_{tag}")
                for q in range(BSHC):
                    msl = mean_tl[:, q * D:(q + 1) * D]
                    sp_t = wp.tile([128, TS], F32, tag="sp_t")
                    nc.scalar.activation(sp_t[:], msl[:, :TS], ACTF.Exp)
                    nc.vector.tensor_scalar_add(sp_t[:], sp_t[:], 1.0)
                    nc.scalar.activation(sp_t[:], sp_t[:], ACTF.Ln)
                    spT = wp.tile([TS, 128], F32, tag="spT")
                    transpose128(spT[:], sp_t[:], 128, TS)
                    stp = pp.tile([128, D], F32, tag="stp")
                    nc.tensor.matmul(out=stp[:], lhsT=spT[:], rhs=lwT[:])
                    std = wp.tile([128, D], F32, tag="std")
                    nc.vector.tensor_add(out=std[:], in0=stp[:],
                                         in1=lb_rep[:])
                    nc.vector.tensor_scalar_add(std[:], std[:], 1e-8)
                    g = gen_n[:, q * D:(q + 1) * D]
                    nc.vector.tensor_tensor(
                        out=g, in0=eps_tl[:, q * D:(q + 1) * D], in1=std[:],
                        op=ALU.mult)
                    nc.vector.tensor_add(out=g, in0=g, in1=msl)
                normalize_rows(gen_n[:], BSHC)
                for q in range(BSHC):
                    transpose128(genT[:, q * 128:(q + 1) * 128],
                                 gen_n[:, q * D:(q + 1) * D], 128, D)
                return gen_n, genT

            u_gen_n, u_gen_T = gen_pipe(ua_sh, eps_u, "gu")
            i_gen_n, i_gen_T = gen_pipe(iap_sh, eps_p, "gi")

            psb = cp.tile([1, 16], F32)
            nc.vector.memset(psb[:], 0.0)

            def part_sum(vec_ap, slot, P=128):
                ps = pp.tile([1, 1], F32, tag="pscal")
                nc.tensor.matmul(out=ps[:], lhsT=vec_ap, rhs=ones_col[:P, :])
                nc.vector.tensor_add(out=psb[:, slot:slot + 1],
                                     in0=psb[:, slot:slot + 1], in1=ps[:])

            def infonce(e1_n, e1_T, e2sh_n, e2_tiles, slot):
                lgs = wp.tile([128, BSHC], F32, tag="lgs")
                for q in range(BSHC):
                    prod = wp.tile([128, D], F32, tag="prod")
                    nc.vector.tensor_tensor(
                        out=prod[:], in0=e1_n[:, q * D:(q + 1) * D],
                        in1=e2sh_n[:, q * D:(q + 1) * D], op=ALU.mult)
                    pdot = wp.tile([128, 1], F32, tag="pdot")
                    nc.vector.tensor_reduce(out=pdot[:], in_=prod[:],
                                            axis=AX.X, op=ALU.add)
                    pex = wp.tile([128, 1], F32, tag="pex")
                    nc.scalar.activation(pex[:], pdot[:], ACTF.Exp,
                                         scale=1.0 / TEMP)
                    nss = wp.tile([128, BC], F32, tag="nss")
                    for ch in range(BC):
                        tl = e2_tiles[ch // BSHC]
                        pcol = (ch % BSHC) * 128
                        zp = pp.tile([128, 128], F32, tag="zneg")
                        nc.tensor.matmul(
                            out=zp[:], lhsT=e1_T[:, q * 128:(q + 1) * 128],
                            rhs=tl[:, pcol:pcol + 128])
                        ju = wp.tile([128, 128], F32, tag="ju")
                        nc.scalar.activation(
                            ju[:], zp[:], ACTF.Exp, scale=1.0 / TEMP,
                            accum_out=nss[:, ch:ch + 1])
                    nsum = wp.tile([128, 1], F32, tag="nsum")
                    nc.vector.tensor_reduce(out=nsum[:], in_=nss[:],
                                            axis=AX.X, op=ALU.add)
                    nc.vector.tensor_scalar_add(nsum[:], nsum[:], 1e-8)
                    rn = wp.tile([128, 1], F32, tag="rng")
                    nc.vector.reciprocal(rn[:], nsum[:])
                    qv = wp.tile([128, 1], F32, tag="qv")
                    nc.vector.tensor_tensor(out=qv[:], in0=pex[:], in1=rn[:],
                                            op=ALU.mult)
                    nc.vector.tensor_scalar_add(qv[:], qv[:], 1e-8)
                    nc.scalar.activation(lgs[:, q:q + 1], qv[:], ACTF.Ln)
                tot = wp.tile([128, 1], F32, tag="lgt")
                nc.vector.tensor_reduce(out=tot[:], in_=lgs[:], axis=AX.X,
                                        op=ALU.add)
                part_sum(tot[:], slot)

            infonce(u_gen_n, u_gen_T, u_i_n, uT_tiles, 2)
            infonce(i_gen_n, i_gen_T, i_i_n, iT_tiles, 3)

            # BPR
            dsc = wp.tile([128, BSHC], F32, tag="dsc")
            for q in range(BSHC):
                pr = wp.tile([128, D], F32, tag="bprp")
                nc.vector.tensor_tensor(
                    out=pr[:], in0=ua_sh[:, q * D:(q + 1) * D],
                    in1=iap_sh[:, q * D:(q + 1) * D], op=ALU.mult)
                ps_ = wp.tile([128, 1], F32, tag="bps")
                nc.vector.tensor_reduce(out=ps_[:], in_=pr[:], axis=AX.X,
                                        op=ALU.add)
                nr = wp.tile([128, D], F32, tag="bprn")
                nc.vector.tensor_tensor(
                    out=nr[:], in0=ua_sh[:, q * D:(q + 1) * D],
                    in1=ian_sh[:, q * D:(q + 1) * D], op=ALU.mult)
                ns_ = wp.tile([128, 1], F32, tag="bns")
                nc.vector.tensor_reduce(out=ns_[:], in_=nr[:], axis=AX.X,
                                        op=ALU.add)
                nc.vector.tensor_sub(out=dsc[:, q:q + 1], in0=ns_[:],
                                     in1=ps_[:])
            spl = wp.tile([128, BSHC], F32, tag="spl")
            nc.scalar.activation(spl[:], dsc[:], ACTF.Exp)
            nc.vector.tensor_scalar_add(spl[:], spl[:], 1.0)
            nc.scalar.activation(spl[:], spl[:], ACTF.Ln)
            bps = wp.tile([128, 1], F32, tag="bpst")
            nc.vector.tensor_reduce(out=bps[:], in_=spl[:], axis=AX.X,
                                    op=ALU.add)
            part_sum(bps[:], 0)

            # KL over own shard (from acc)
            KW = 8
            klcols = cp.tile([128, PC], F32)
            for g in range(math.ceil(PC / KW)):
                w0 = g * KW
                W = min(KW, PC - w0)
                mean_g = acc[:, w0 * D:(w0 + W) * D]
                spg = wp.tile([128, KW * TS], F32, tag="spg")
                nc.scalar.activation(
                    spg[:, :W * TS].rearrange("p (c d) -> p c d", d=TS),
                    acc[:, w0 * D:].rearrange(
                        "p (c d) -> p c d", d=D)[:, 0:W, 0:TS],
                    ACTF.Exp)
                nc.vector.tensor_scalar_add(spg[:, :W * TS], spg[:, :W * TS],
                                            1.0)
                nc.scalar.activation(spg[:, :W * TS], spg[:, :W * TS],
                                     ACTF.Ln)
                stdg = wp.tile([128, KW * D], F32, tag="stdg")
                for w in range(W):
                    spT = wp.tile([TS, 128], F32, tag="spTk")
                    transpose128(spT[:], spg[:, w * TS:(w + 1) * TS], 128, TS)
                    stp = pp.tile([128, D], F32, tag="stp")
                    nc.tensor.matmul(out=stp[:], lhsT=spT[:], rhs=lwT[:])
                    sw = stdg[:, w * D:(w + 1) * D]
                    nc.vector.tensor_add(out=sw, in0=stp[:], in1=lb_rep[:])
                    nc.vector.tensor_scalar_add(sw, sw, 1e-8)
                m2 = wp.tile([128, KW * D], F32, tag="m2")
                nc.scalar.activation(m2[:, :W * D], mean_g, ACTF.Square)
                exg = wp.tile([128, KW * D], F32, tag="exg")
                nc.scalar.activation(exg[:, :W * D], stdg[:, :W * D],
                                     ACTF.Exp, scale=2.0)
                t1 = wp.tile([128, KW * D], F32, tag="t1")
                nc.scalar.activation(t1[:, :W * D], stdg[:, :W * D],
                                     ACTF.Copy, bias=0.0, scale=2.0)
                nc.vector.tensor_scalar_add(t1[:, :W * D], t1[:, :W * D], 1.0)
                nc.vector.tensor_sub(out=t1[:, :W * D], in0=t1[:, :W * D],
                                     in1=m2[:, :W * D])
                nc.vector.tensor_sub(out=t1[:, :W * D], in0=t1[:, :W * D],
                                     in1=exg[:, :W * D])
                nc.vector.tensor_reduce(
                    out=klcols[:, w0:w0 + W],
                    in_=t1[:, :W * D].rearrange("p (c d) -> p c d", d=D),
                    axis=AX.X, op=ALU.add)
            nc.vector.tensor_tensor(out=klcols[:], in0=klcols[:],
                                    in1=kmask[:], op=ALU.mult)
            ktot = wp.tile([128, 1], F32, tag="ktot")
            nc.vector.tensor_reduce(out=ktot[:], in_=klcols[:], axis=AX.X,
                                    op=ALU.add)
            part_sum(ktot[:], 1)

            nc.sync.dma_start(out=partials[:], in_=psb[:])

    return nc


# --------------------------------------------------------------------------
# entry
# --------------------------------------------------------------------------

def prepare(inputs, c, for_sim=False):
    """Returns (nc, in_maps)."""
    NC, D, SPAD = c["NC"], c["D"], c["SPAD"]
    per_core, perm_pos, trow_vec, ZROW = host_prep(inputs, c)

    users0 = np.asarray(inputs["users"]).astype(np.int64)
    pos0 = np.asarray(inputs["pos_items"]).astype(np.int64)
    neg0 = np.asarray(inputs["neg_items"]).astype(np.int64)
    N_USERS, B, BSH, BC, BSHC = (c["N_USERS"], c["B"], c["BSH"], c["BCOLS"],
                                 c["BSHC"])
    SENT = 1 << 20

    # ---- 2-phase batch tables ----
    # global batch node list: users | pos | neg (original order)
    nodes = np.concatenate([users0, N_USERS + pos0, N_USERS + neg0])
    gpos = perm_pos[nodes]                      # perm position (k*SPAD+j)
    owner = gpos // SPAD
    localj = gpos % SPAD
    # local cyclic row in accD: row = j (accD is [SPAD, D] cyclic c*128+p ->
    # row j directly since accD[(c p)] with j = c*128+p)
    # compact position assignment per core, in global slot order
    NBATCH = len(nodes)
    own_lists = [np.where(owner == k)[0] for k in range(NC)]
    max_own = max(len(x) for x in own_lists)
    SK = math.ceil(max_own / 128)
    c["SK"] = SK
    # compact position of global slot i on its owner core
    compact_pos = np.empty(NBATCH, dtype=np.int64)
    idx_own_cores = []
    for k in range(NC):
        lst = own_lists[k]
        compact_pos[lst] = np.arange(len(lst))
        col = np.full(SK * 128, SENT, dtype=np.int64)
        col[:len(lst)] = localj[lst]
        # gather op q covers positions q*128..q*128+127; position i ->
        # partition i%128 in the [128, D] out tile
        idx_own_cores.append(col.reshape(SK, 128).T.astype(np.int32))
    # assembly index: batchT row of global slot i = owner*SK*128 + compact
    asm_row = owner * (SK * 128) + compact_pos

    nc_prog = None  # built after SK known

    def cycb_rows(rows):
        m = len(rows) // 128
        return rows.reshape(m, 128).T.astype(np.int32)

    eps_np = np.asarray(inputs["eps"], dtype=np.float32)
    ui_np = np.asarray(inputs["user_intent"], dtype=np.float32)
    ii_np = np.asarray(inputs["item_intent"], dtype=np.float32)
    lw_np = np.asarray(inputs["lin_w"], dtype=np.float32)
    lb_rep = np.tile(np.asarray(inputs["lin_b"],
                                dtype=np.float32)[None, :], (128, 1))

    in_maps = []
    for k in range(NC):
        rot = np.roll(np.arange(B), -k * BSH)
        # assembled columns: IU (users rotated, full B) | IP (pos rotated,
        # full B) | NS (neg rotated, shard only)
        iu_rows = asm_row[rot]                       # users
        ip_rows = asm_row[B + rot]                   # pos
        ns_rows = asm_row[2 * B + rot[:BSH]]         # neg shard
        idx_asm = np.concatenate([
            cycb_rows(iu_rows), cycb_rows(ip_rows), cycb_rows(ns_rows),
        ], axis=1)
        # eps pre-gather: users shard + pos shard (original node indices)
        eu = eps_np[users0[rot][:BSH]]               # [BSH, D]
        ep = eps_np[N_USERS + pos0[rot][:BSH]]
        def cycd(x):
            return x.reshape(BSHC, 128, D).transpose(1, 0, 2).reshape(
                128, BSHC * D)
        eps_b = np.concatenate([cycd(eu), cycd(ep)], axis=1)
        pk = per_core[k]
        in_maps.append(dict(
            ego_perm=pk["ego_perm"], idx_spmm=pk["idx_spmm"],
            dinv=pk["dinv"], dinv2=pk["dinv2"], kmask=pk["kmask"],
            idx_own=idx_own_cores[k], idx_asm=idx_asm, eps_b=eps_b,
            user_intent=ui_np, item_intent=ii_np, lin_w=lw_np,
            lin_b_rep=lb_rep))

    nc_prog = build_bass(c)
    if not for_sim:
        split_multi_waits(nc_prog)
    return nc_prog, in_maps


def combine(results, c, inputs):
    NC, B, N = c["NC"], c["B"], c["N"]
    P = np.stack([np.asarray(results[k]["partials"][0], dtype=np.float64)
                  for k in range(NC)])
    bpr = P[:, 0].sum() / B
    kl = c["KL_REG"] * (-0.5 * P[:, 1].sum()) / N
    gen_loss = np.float32(bpr + kl)
    cl_loss = np.float32(c["SSL_REG"] * (-(P[:, 2].sum()) - P[:, 3].sum()) / B)

    users = np.asarray(inputs["users"]).astype(np.int64)
    pos = np.asarray(inputs["pos_items"]).astype(np.int64)
    neg = np.asarray(inputs["neg_items"]).astype(np.int64)
    ue = np.asarray(inputs["user_emb"], dtype=np.float64)
    ie = np.asarray(inputs["item_emb"], dtype=np.float64)
    ui = np.asarray(inputs["user_intent"], dtype=np.float64)
    ii = np.asarray(inputs["item_intent"], dtype=np.float64)
    emb_loss = np.float32(c["EMB_REG"] * (
        np.sum(np.square(ue[users])) + np.sum(np.square(ie[pos]))
        + np.sum(np.square(ie[neg]))))
    int_loss = np.float32(c["INT_REG"] * (np.sum(np.square(ui))
                                          + np.sum(np.square(ii))))
    return (gen_loss, cl_loss, emb_loss, int_loss)


def kernel(**inputs):
    c = derive(default_cfg())
    nc, in_maps = prepare(inputs, c)
    res = run_bass_kernel_spmd(nc, in_maps, list(range(c["NC"])))
    return combine(res.results, c, inputs)
